# revision 17
# baseline (speedup 1.0000x reference)
"""Self-contained TRN2 Bass kernel for NeuralFSM message passing.

kernel(s0, edge_index, T) -> [100000, 8] float32, running 20 FSM iterations
on 8 NeuronCores via concourse/bass (SPMD, node-partitioned).

Algorithm: states are one-hot; threshold(segment_sum) == bitwise OR of
neighbor state bits. Per node keep a byte b = 1<<state packed 4-per-uint32
in an SBUF-resident table replicated across partitions; per iteration
ap_gather the static edge-slot streams, shift-extract the source byte,
OR-reduce per destination (uniform padded slot count per chunk), look up the
256x8 FSM transition via two small gathers, AllGather the rebuilt table
words across the 8 cores, and re-broadcast.
"""
import os
import sys

import numpy as np

for _p in ("/opt/trn_rl_repo", "/root/.axon_site/_ro/trn_rl_repo", "/root/.axon_site"):
    if os.path.isdir(_p) and _p not in sys.path:
        sys.path.append(_p)

N_REAL = 100000
S = 8
NC = 8
P = 128
CORES = 8
RPC = 16
JROWS = 98
NODES_CORE = RPC * JROWS          # 1568
NDST_NC = P * JROWS               # 12544
NTOT = NC * NDST_NC               # 100352
WORDS_CORE = NODES_CORE // 4      # 392
WORDS_NC = WORDS_CORE * CORES     # 3136
NWORDS = 1 + NC * WORDS_NC        # 25089
T2SIZE = 2049
ITERS = 20
CHUNK_BUDGET = 3584

LAST_EXEC_NS = None


class _Layout:
    def __init__(self, edge_index):
        src_all = edge_index[0].astype(np.int64)
        dst_all = edge_index[1].astype(np.int64)
        deg = np.bincount(dst_all, minlength=N_REAL)
        L_node = np.maximum(1, -(-deg // 4)) * 4
        order = np.argsort(-L_node, kind="stable")
        node_of_z = np.full(NTOT, -1, dtype=np.int64)
        node_of_z[:N_REAL] = order
        z = np.arange(NTOT)
        self.j_of_z = z // (NC * P)
        self.nc_of_z = (z % (NC * P)) // P
        self.p_of_z = z % P
        self.node_of_z = node_of_z
        z_of_node = np.full(N_REAL, -1, dtype=np.int64)
        z_of_node[order] = np.arange(N_REAL)
        self.z_of_node = z_of_node

        Lz = np.zeros(NTOT, dtype=np.int64)
        Lz[:N_REAL] = L_node[order]
        self.L_row = np.maximum(1, Lz.reshape(JROWS, NC * P).max(axis=1) // 4) * 4

        c_of_z = self.p_of_z // RPC
        r_of_z = self.p_of_z % RPC
        self.word_of_z = 1 + self.nc_of_z * WORDS_NC + c_of_z * WORDS_CORE \
            + 4 * self.j_of_z + r_of_z // 4
        self.lane_of_z = r_of_z % 4

        chunks = []
        j = 0
        while j < JROWS:
            L = int(self.L_row[j])
            j2 = j
            while j2 < JROWS and self.L_row[j2] == L:
                j2 += 1
            max_jr = max(1, CHUNK_BUDGET // (RPC * L))
            while j < j2:
                jr = min(max_jr, j2 - j)
                chunks.append((j, jr, L))
                j += jr
        self.chunks = chunks
        self.slots_per_core = int(sum(RPC * jr * L for (_, jr, L) in chunks))

        # edges grouped by dst placement
        zdst = z_of_node[dst_all]
        eorder = np.argsort(zdst, kind="stable")
        src_by_z = src_all[eorder]
        zsorted = zdst[eorder]
        starts = np.searchsorted(zsorted, np.arange(NTOT))
        ends = np.searchsorted(zsorted, np.arange(NTOT) + 1)
        src_w = self.word_of_z[z_of_node[src_by_z]].astype(np.int16)
        src_sh = (8 * self.lane_of_z[z_of_node[src_by_z]]).astype(np.uint8)

        Tc = self.slots_per_core
        # stream position of slot s of dst z: per (nc,c): t = chunk_off + ((jj*16+r)*L) + s
        # build per-z slot base in stream, then scatter srcs
        row_off = np.zeros(JROWS, dtype=np.int64)     # chunk_stream_off + jj*16*L
        row_L = np.zeros(JROWS, dtype=np.int64)
        to = 0
        for (j0, jr, L) in chunks:
            for jj in range(jr):
                row_off[j0 + jj] = to + jj * RPC * L
                row_L[j0 + jj] = L
            to += RPC * jr * L
        base_z = row_off[self.j_of_z] + (r_of_z) * row_L[self.j_of_z]
        # expand: slot position for each sorted edge
        cnt = ends - starts
        e_z = np.repeat(np.arange(NTOT), cnt)
        within = np.arange(len(src_by_z)) - np.repeat(starts, cnt)
        t_pos = base_z[e_z] + within
        stream_idx = np.zeros((NC, CORES, Tc), dtype=np.int16)
        stream_sh = np.zeros((NC, CORES, Tc), dtype=np.uint8)
        stream_idx[self.nc_of_z[e_z], c_of_z[e_z], t_pos] = src_w
        stream_sh[self.nc_of_z[e_z], c_of_z[e_z], t_pos] = src_sh
        self.stream_sh = stream_sh

        self.idx_wrapped = np.zeros((NC, P, Tc // RPC), dtype=np.int16)
        for nc_ in range(NC):
            for c in range(CORES):
                st = stream_idx[nc_, c]
                self.idx_wrapped[nc_, c * RPC:(c + 1) * RPC, :] = \
                    st.reshape(Tc // RPC, RPC).T


def _build_kernel(chunks, slots_per_core, iters=ITERS):
    from concourse import bacc, tile, mybir

    u32 = mybir.dt.uint32
    u8 = mybir.dt.uint8
    i16 = mybir.dt.int16
    Alu = mybir.AluOpType
    X = mybir.AxisListType.X

    T_core = slots_per_core
    TP = T_core // 16
    CH = max(RPC * jr * L for (_, jr, L) in chunks)

    nc = bacc.Bacc("TRN2", target_bir_lowering=False, debug=False,
                   enable_asserts=True, num_devices=NC)
    t_idx = nc.dram_tensor("t_idx", [P, TP], i16, kind="ExternalInput")
    t_shift = nc.dram_tensor("t_shift", [P, T_core], u32, kind="ExternalInput")
    t_W0 = nc.dram_tensor("t_W0", [P, NWORDS], u32, kind="ExternalInput")
    t_q0 = nc.dram_tensor("t_q0", [P, NODES_CORE], u32, kind="ExternalInput")
    t_T2N1 = nc.dram_tensor("t_T2N1", [P, T2SIZE], u32, kind="ExternalInput")
    t_T2L0 = nc.dram_tensor("t_T2L0", [P, T2SIZE], u32, kind="ExternalInput")
    t_lane = nc.dram_tensor("t_lane", [P, RPC], u32, kind="ExternalInput")
    t_m16 = nc.dram_tensor("t_m16", [P, RPC], u32, kind="ExternalInput")
    t_qout = nc.dram_tensor("t_qout", [P, NODES_CORE], u32, kind="ExternalOutput")

    with tile.TileContext(nc) as tc:
        with tc.tile_pool(name="dram", bufs=2, space="DRAM") as dram, \
             tc.tile_pool(name="per", bufs=1) as per, \
             tc.tile_pool(name="chk", bufs=2) as chk, \
             tc.tile_pool(name="sh32", bufs=1) as sh32p, \
             tc.tile_pool(name="qq", bufs=2) as qq:
            W = per.tile([P, NWORDS], u32)
            idx = per.tile([P, TP], i16)
            T2N1 = per.tile([P, T2SIZE], u32)
            T2L0 = per.tile([P, T2SIZE], u32)
            lane = per.tile([P, RPC], u32)
            mask = per.tile([P, NODES_CORE], u32)
            tmp = per.tile([P, NODES_CORE + 16], u32)  # +16: strided-read footprint guard
            idx16 = per.tile([P, JROWS], i16)
            words = per.tile([P, WORDS_CORE], u32)
            m16 = per.tile([P, RPC], u32)
            idxw = per.tile([P, JROWS], u32)

            nc.gpsimd.memset(tmp[:], 0)
            nc.sync.dma_start(out=W[:], in_=t_W0[:])
            nc.sync.dma_start(out=idx[:], in_=t_idx[:])
            nc.sync.dma_start(out=T2N1[:], in_=t_T2N1[:])
            nc.sync.dma_start(out=T2L0[:], in_=t_T2L0[:])
            nc.sync.dma_start(out=lane[:], in_=t_lane[:])
            nc.sync.dma_start(out=m16[:], in_=t_m16[:])
            q = qq.tile([P, NODES_CORE], u32, tag="q")
            nc.sync.dma_start(out=q[:], in_=t_q0[:])

            for it in range(iters):
                mo = 0
                to = 0
                for (j0, jr, L) in chunks:
                    n = RPC * jr * L
                    jr16 = RPC * jr
                    gout = chk.tile([P, CH], u32, tag="gout")
                    nc.gpsimd.ap_gather(
                        out_ap=gout[:, :n], in_ap=W[:],
                        idxs_ap=idx[:, to // 16:(to + n) // 16],
                        channels=P, num_elems=NWORDS, d=1, num_idxs=n)
                    shu32 = sh32p.tile([P, CH], u32, tag="shu32")
                    nc.sync.dma_start(out=shu32[:, :n], in_=t_shift[:, to:to + n])
                    nc.vector.tensor_tensor(
                        out=gout[:, :n], in0=gout[:, :n], in1=shu32[:, :n],
                        op=Alu.logical_shift_right)
                    nc.vector.tensor_reduce(
                        out=mask[:, mo:mo + jr16],
                        in_=gout[:, :n].rearrange("p (a b) -> p a b", b=L),
                        axis=X, op=Alu.bitwise_or)
                    mo += jr16
                    to += n
                assert mo == NODES_CORE and to == T_core

                nc.vector.tensor_scalar(
                    out=tmp[:, :NODES_CORE], in0=mask[:], scalar1=0xFF, scalar2=3,
                    op0=Alu.bitwise_and, op1=Alu.logical_shift_left)
                nc.vector.tensor_tensor(out=tmp[:, :NODES_CORE], in0=tmp[:, :NODES_CORE], in1=q[:],
                                        op=Alu.add)
                # wrapped select: idx16[p, j] = tmp[p, 16*j + p%16]
                nc.vector.tensor_tensor(
                    out=tmp[:, :NODES_CORE], in0=tmp[:, :NODES_CORE],
                    in1=m16[:, None, :].broadcast_to([P, JROWS, RPC]),
                    op=Alu.bitwise_and)
                nc.vector.tensor_reduce(
                    out=idxw[:],
                    in_=tmp[:, :NODES_CORE].rearrange("p (a b) -> p a b", b=RPC),
                    axis=X, op=Alu.bitwise_or)
                nc.vector.tensor_copy(idx16[:], idxw[:])
                qn = qq.tile([P, NODES_CORE], u32, tag="q")
                if it == iters - 1:
                    nc.gpsimd.ap_gather(out_ap=qn[:], in_ap=T2N1[:],
                                        idxs_ap=idx16[:], channels=P,
                                        num_elems=T2SIZE, d=1,
                                        num_idxs=NODES_CORE)
                q = qn
                if it < iters - 1:
                    nc.gpsimd.ap_gather(out_ap=tmp[:, :NODES_CORE], in_ap=T2L0[:],
                                        idxs_ap=idx16[:], channels=P,
                                        num_elems=T2SIZE, d=1,
                                        num_idxs=NODES_CORE)
                    nc.vector.tensor_tensor(
                        out=tmp[:, :NODES_CORE].rearrange("p (a b) -> p a b", b=RPC),
                        in0=tmp[:, :NODES_CORE].rearrange("p (a b) -> p a b", b=RPC),
                        in1=lane[:, None, :].broadcast_to([P, JROWS, RPC]),
                        op=Alu.logical_shift_left)
                    nc.vector.tensor_reduce(
                        out=words[:], in_=tmp[:, :NODES_CORE].rearrange("p (a b) -> p a b", b=4),
                        axis=X, op=Alu.bitwise_or)
                    dwords = dram.tile([1, WORDS_NC], u32, tag="dw")
                    dgath = dram.tile([1, NC * WORDS_NC], u32, tag="dg")
                    nc.sync.dma_start(out=dwords[:], in_=words[0::16, :])
                    nc.gpsimd.collective_compute(
                        "AllGather", Alu.bypass,
                        replica_groups=[list(range(NC))],
                        ins=[dwords.opt()], outs=[dgath.opt()])
                    # T2N gather emitted after the collective trigger so it
                    # overlaps the AllGather on the TOPSP engines
                    nc.gpsimd.ap_gather(out_ap=qn[:], in_ap=T2N1[:],
                                        idxs_ap=idx16[:], channels=P,
                                        num_elems=T2SIZE, d=1,
                                        num_idxs=NODES_CORE)
                    nc.sync.dma_start(
                        out=W[:, 1:],
                        in_=dgath[0:1, :].broadcast_to([P, NC * WORDS_NC]))
            nc.sync.dma_start(out=t_qout[:], in_=q[:])
    nc.compile()
    return nc


def _device_inputs(lay, s0, T):
    ns_tab = np.argmax(np.asarray(T), axis=2).astype(np.uint32)  # [256, 8]
    flat = ns_tab.reshape(-1)  # idx-1 = mask*8 + state
    T2N1 = np.zeros(T2SIZE, dtype=np.uint32)
    T2N1[1:] = flat + 1
    T2L0 = np.zeros(T2SIZE, dtype=np.uint32)
    T2L0[1:] = (1 << flat).astype(np.uint32)

    st_node = np.argmax(np.asarray(s0), axis=1).astype(np.uint32)
    st_z = np.zeros(NTOT, dtype=np.uint32)
    st_z[:N_REAL] = st_node[lay.node_of_z[:N_REAL]]
    W0 = np.zeros(NWORDS, dtype=np.uint32)
    byte = (1 << st_z).astype(np.uint32) << (8 * lay.lane_of_z)
    np.bitwise_or.at(W0, lay.word_of_z, byte)

    stg = np.zeros((NC, P, JROWS), dtype=np.uint32)
    stg[lay.nc_of_z, lay.p_of_z, lay.j_of_z] = st_z
    lanecode = np.broadcast_to(((np.arange(RPC) % 4) * 8).astype(np.uint32), (P, RPC)).copy()
    m16 = np.zeros((P, RPC), dtype=np.uint32)
    m16[np.arange(P), np.arange(P) % RPC] = 0xFFFFFFFF
    W0b = np.broadcast_to(W0, (P, NWORDS)).copy()
    T2N1b = np.broadcast_to(T2N1, (P, T2SIZE)).copy()
    T2L0b = np.broadcast_to(T2L0, (P, T2SIZE)).copy()

    in_maps = []
    for nci in range(NC):
        q0 = np.zeros((P, NODES_CORE), dtype=np.uint32)
        shf = np.zeros((P, lay.slots_per_core), dtype=np.uint32)
        for c in range(CORES):
            sgrid = stg[nci, c * RPC:(c + 1) * RPC, :]
            q0[c * RPC:(c + 1) * RPC, :] = (sgrid.T.reshape(-1) + 1)[None, :]
            shf[c * RPC:(c + 1) * RPC, :] = lay.stream_sh[nci, c][None, :]
        in_maps.append({
            "t_idx": lay.idx_wrapped[nci],
            "t_shift": shf,
            "t_W0": W0b,
            "t_q0": q0,
            "t_T2N1": T2N1b,
            "t_T2L0": T2L0b,
            "t_lane": lanecode,
            "t_m16": m16,
        })
    return in_maps


def _decode(lay, results):
    stg = np.zeros((NC, P, JROWS), dtype=np.uint32)
    for nci in range(NC):
        qout = results[nci]["t_qout"]
        for c in range(CORES):
            stg[nci, c * RPC:(c + 1) * RPC, :] = \
                qout[c * RPC, :].reshape(JROWS, RPC).T
    st_z = stg[lay.nc_of_z[:N_REAL], lay.p_of_z[:N_REAL],
               lay.j_of_z[:N_REAL]].astype(np.int64) - 1
    st_node = np.zeros(N_REAL, dtype=np.int64)
    st_node[lay.node_of_z[:N_REAL]] = st_z
    out = np.zeros((N_REAL, S), dtype=np.float32)
    out[np.arange(N_REAL), st_node] = 1.0
    return out


def kernel(s0, edge_index, T):
    global LAST_EXEC_NS
    from concourse import bass_utils

    s0 = np.asarray(s0)
    edge_index = np.asarray(edge_index)
    Tn = np.asarray(T)
    lay = _Layout(edge_index)
    nc = _build_kernel(lay.chunks, lay.slots_per_core)
    in_maps = _device_inputs(lay, s0, Tn)
    trace = os.environ.get("BASS_FSM_TRACE", "0") == "1"
    res = bass_utils.run_bass_kernel_spmd(
        nc, in_maps, core_ids=list(range(NC)), trace=trace)
    LAST_EXEC_NS = res.exec_time_ns
    return _decode(lay, res.results).astype(s0.dtype)


# revision 18
# speedup vs baseline: 1.0021x; 1.0021x over previous
"""Self-contained TRN2 Bass kernel for NeuralFSM message passing.

kernel(s0, edge_index, T) -> [100000, 8] float32, running 20 FSM iterations
on 8 NeuronCores via concourse/bass (SPMD, node-partitioned).

Algorithm: states are one-hot; threshold(segment_sum) == bitwise OR of
neighbor state bits. Per node keep a byte b = 1<<state packed 4-per-uint32
in an SBUF-resident table replicated across partitions; per iteration
ap_gather the static edge-slot streams, shift-extract the source byte,
OR-reduce per destination (uniform padded slot count per chunk), look up the
256x8 FSM transition via two small gathers, AllGather the rebuilt table
words across the 8 cores, and re-broadcast.
"""
import os
import sys

import numpy as np

for _p in ("/opt/trn_rl_repo", "/root/.axon_site/_ro/trn_rl_repo", "/root/.axon_site"):
    if os.path.isdir(_p) and _p not in sys.path:
        sys.path.append(_p)

N_REAL = 100000
S = 8
NC = 8
P = 128
CORES = 8
RPC = 16
JROWS = 98
NODES_CORE = RPC * JROWS          # 1568
NDST_NC = P * JROWS               # 12544
NTOT = NC * NDST_NC               # 100352
WORDS_CORE = NODES_CORE // 4      # 392
WORDS_NC = WORDS_CORE * CORES     # 3136
NWORDS = 1 + NC * WORDS_NC        # 25089
T2SIZE = 2049
ITERS = 20
CHUNK_BUDGET = 3584

LAST_EXEC_NS = None


class _Layout:
    def __init__(self, edge_index):
        src_all = edge_index[0].astype(np.int64)
        dst_all = edge_index[1].astype(np.int64)
        deg = np.bincount(dst_all, minlength=N_REAL)
        L_node = np.maximum(1, -(-deg // 4)) * 4
        order = np.argsort(-L_node, kind="stable")
        node_of_z = np.full(NTOT, -1, dtype=np.int64)
        node_of_z[:N_REAL] = order
        z = np.arange(NTOT)
        self.j_of_z = z // (NC * P)
        self.nc_of_z = (z % (NC * P)) // P
        self.p_of_z = z % P
        self.node_of_z = node_of_z
        z_of_node = np.full(N_REAL, -1, dtype=np.int64)
        z_of_node[order] = np.arange(N_REAL)
        self.z_of_node = z_of_node

        Lz = np.zeros(NTOT, dtype=np.int64)
        Lz[:N_REAL] = L_node[order]
        self.L_row = np.maximum(1, Lz.reshape(JROWS, NC * P).max(axis=1) // 4) * 4

        c_of_z = self.p_of_z // RPC
        r_of_z = self.p_of_z % RPC
        self.word_of_z = 1 + self.nc_of_z * WORDS_NC + c_of_z * WORDS_CORE \
            + 4 * self.j_of_z + r_of_z // 4
        self.lane_of_z = r_of_z % 4

        chunks = []
        j = 0
        while j < JROWS:
            L = int(self.L_row[j])
            j2 = j
            while j2 < JROWS and self.L_row[j2] == L:
                j2 += 1
            max_jr = max(1, CHUNK_BUDGET // (RPC * L))
            while j < j2:
                jr = min(max_jr, j2 - j)
                chunks.append((j, jr, L))
                j += jr
        self.chunks = chunks
        self.slots_per_core = int(sum(RPC * jr * L for (_, jr, L) in chunks))

        # edges grouped by dst placement
        zdst = z_of_node[dst_all]
        eorder = np.argsort(zdst, kind="stable")
        src_by_z = src_all[eorder]
        zsorted = zdst[eorder]
        starts = np.searchsorted(zsorted, np.arange(NTOT))
        ends = np.searchsorted(zsorted, np.arange(NTOT) + 1)
        src_w = self.word_of_z[z_of_node[src_by_z]].astype(np.int16)
        src_sh = (8 * self.lane_of_z[z_of_node[src_by_z]]).astype(np.uint8)

        Tc = self.slots_per_core
        # stream position of slot s of dst z: per (nc,c): t = chunk_off + ((jj*16+r)*L) + s
        # build per-z slot base in stream, then scatter srcs
        row_off = np.zeros(JROWS, dtype=np.int64)     # chunk_stream_off + jj*16*L
        row_L = np.zeros(JROWS, dtype=np.int64)
        to = 0
        for (j0, jr, L) in chunks:
            for jj in range(jr):
                row_off[j0 + jj] = to + jj * RPC * L
                row_L[j0 + jj] = L
            to += RPC * jr * L
        base_z = row_off[self.j_of_z] + (r_of_z) * row_L[self.j_of_z]
        # expand: slot position for each sorted edge
        cnt = ends - starts
        e_z = np.repeat(np.arange(NTOT), cnt)
        within = np.arange(len(src_by_z)) - np.repeat(starts, cnt)
        t_pos = base_z[e_z] + within
        stream_idx = np.zeros((NC, CORES, Tc), dtype=np.int16)
        stream_sh = np.zeros((NC, CORES, Tc), dtype=np.uint8)
        stream_idx[self.nc_of_z[e_z], c_of_z[e_z], t_pos] = src_w
        stream_sh[self.nc_of_z[e_z], c_of_z[e_z], t_pos] = src_sh
        self.stream_sh = stream_sh

        self.idx_wrapped = np.zeros((NC, P, Tc // RPC), dtype=np.int16)
        for nc_ in range(NC):
            for c in range(CORES):
                st = stream_idx[nc_, c]
                self.idx_wrapped[nc_, c * RPC:(c + 1) * RPC, :] = \
                    st.reshape(Tc // RPC, RPC).T


def _build_kernel(chunks, slots_per_core, iters=ITERS):
    from concourse import bacc, tile, mybir

    u32 = mybir.dt.uint32
    u8 = mybir.dt.uint8
    i16 = mybir.dt.int16
    Alu = mybir.AluOpType
    X = mybir.AxisListType.X

    T_core = slots_per_core
    TP = T_core // 16
    CH = max(RPC * jr * L for (_, jr, L) in chunks)

    nc = bacc.Bacc("TRN2", target_bir_lowering=False, debug=False,
                   enable_asserts=True, num_devices=NC)
    t_idx = nc.dram_tensor("t_idx", [P, TP], i16, kind="ExternalInput")
    t_shift = nc.dram_tensor("t_shift", [P, T_core], u8, kind="ExternalInput")
    t_W0 = nc.dram_tensor("t_W0", [P, NWORDS], u32, kind="ExternalInput")
    t_q0 = nc.dram_tensor("t_q0", [P, NODES_CORE], u32, kind="ExternalInput")
    t_T2N1 = nc.dram_tensor("t_T2N1", [P, T2SIZE], u32, kind="ExternalInput")
    t_T2L0 = nc.dram_tensor("t_T2L0", [P, T2SIZE], u32, kind="ExternalInput")
    t_lane = nc.dram_tensor("t_lane", [P, RPC], u32, kind="ExternalInput")
    t_m16 = nc.dram_tensor("t_m16", [P, RPC], u32, kind="ExternalInput")
    t_qout = nc.dram_tensor("t_qout", [P, NODES_CORE], u32, kind="ExternalOutput")

    with tile.TileContext(nc) as tc:
        with tc.tile_pool(name="dram", bufs=2, space="DRAM") as dram, \
             tc.tile_pool(name="per", bufs=1) as per, \
             tc.tile_pool(name="chk", bufs=2) as chk, \
             tc.tile_pool(name="sh32", bufs=1) as sh32p, \
             tc.tile_pool(name="qq", bufs=2) as qq:
            W = per.tile([P, NWORDS], u32)
            idx = per.tile([P, TP], i16)
            T2N1 = per.tile([P, T2SIZE], u32)
            T2L0 = per.tile([P, T2SIZE], u32)
            lane = per.tile([P, RPC], u32)
            mask = per.tile([P, NODES_CORE], u32)
            tmp = per.tile([P, NODES_CORE + 16], u32)  # +16: strided-read footprint guard
            idx16 = per.tile([P, JROWS], i16)
            words = per.tile([P, WORDS_CORE], u32)
            m16 = per.tile([P, RPC], u32)
            idxw = per.tile([P, JROWS], u32)

            nc.gpsimd.memset(tmp[:], 0)
            nc.sync.dma_start(out=W[:], in_=t_W0[:])
            nc.sync.dma_start(out=idx[:], in_=t_idx[:])
            nc.sync.dma_start(out=T2N1[:], in_=t_T2N1[:])
            nc.sync.dma_start(out=T2L0[:], in_=t_T2L0[:])
            nc.sync.dma_start(out=lane[:], in_=t_lane[:])
            nc.sync.dma_start(out=m16[:], in_=t_m16[:])
            q = qq.tile([P, NODES_CORE], u32, tag="q")
            nc.sync.dma_start(out=q[:], in_=t_q0[:])

            for it in range(iters):
                mo = 0
                to = 0
                for (j0, jr, L) in chunks:
                    n = RPC * jr * L
                    jr16 = RPC * jr
                    gout = chk.tile([P, CH], u32, tag="gout")
                    nc.gpsimd.ap_gather(
                        out_ap=gout[:, :n], in_ap=W[:],
                        idxs_ap=idx[:, to // 16:(to + n) // 16],
                        channels=P, num_elems=NWORDS, d=1, num_idxs=n)
                    shu8 = chk.tile([P, CH], u8, tag="shu8")
                    nc.sync.dma_start(out=shu8[:, :n], in_=t_shift[:, to:to + n])
                    shu32 = sh32p.tile([P, CH], u32, tag="shu32")
                    nc.vector.tensor_copy(shu32[:, :n], shu8[:, :n])
                    nc.vector.tensor_tensor(
                        out=gout[:, :n], in0=gout[:, :n], in1=shu32[:, :n],
                        op=Alu.logical_shift_right)
                    nc.vector.tensor_reduce(
                        out=mask[:, mo:mo + jr16],
                        in_=gout[:, :n].rearrange("p (a b) -> p a b", b=L),
                        axis=X, op=Alu.bitwise_or)
                    mo += jr16
                    to += n
                assert mo == NODES_CORE and to == T_core

                nc.vector.tensor_scalar(
                    out=tmp[:, :NODES_CORE], in0=mask[:], scalar1=0xFF, scalar2=3,
                    op0=Alu.bitwise_and, op1=Alu.logical_shift_left)
                nc.vector.tensor_tensor(out=tmp[:, :NODES_CORE], in0=tmp[:, :NODES_CORE], in1=q[:],
                                        op=Alu.add)
                # wrapped select: idx16[p, j] = tmp[p, 16*j + p%16]
                nc.vector.tensor_tensor(
                    out=tmp[:, :NODES_CORE], in0=tmp[:, :NODES_CORE],
                    in1=m16[:, None, :].broadcast_to([P, JROWS, RPC]),
                    op=Alu.bitwise_and)
                nc.vector.tensor_reduce(
                    out=idxw[:],
                    in_=tmp[:, :NODES_CORE].rearrange("p (a b) -> p a b", b=RPC),
                    axis=X, op=Alu.bitwise_or)
                nc.vector.tensor_copy(idx16[:], idxw[:])
                qn = qq.tile([P, NODES_CORE], u32, tag="q")
                if it == iters - 1:
                    nc.gpsimd.ap_gather(out_ap=qn[:], in_ap=T2N1[:],
                                        idxs_ap=idx16[:], channels=P,
                                        num_elems=T2SIZE, d=1,
                                        num_idxs=NODES_CORE)
                q = qn
                if it < iters - 1:
                    nc.gpsimd.ap_gather(out_ap=tmp[:, :NODES_CORE], in_ap=T2L0[:],
                                        idxs_ap=idx16[:], channels=P,
                                        num_elems=T2SIZE, d=1,
                                        num_idxs=NODES_CORE)
                    nc.vector.tensor_tensor(
                        out=tmp[:, :NODES_CORE].rearrange("p (a b) -> p a b", b=RPC),
                        in0=tmp[:, :NODES_CORE].rearrange("p (a b) -> p a b", b=RPC),
                        in1=lane[:, None, :].broadcast_to([P, JROWS, RPC]),
                        op=Alu.logical_shift_left)
                    nc.vector.tensor_reduce(
                        out=words[:], in_=tmp[:, :NODES_CORE].rearrange("p (a b) -> p a b", b=4),
                        axis=X, op=Alu.bitwise_or)
                    dwords = dram.tile([1, WORDS_NC], u32, tag="dw")
                    dgath = dram.tile([1, NC * WORDS_NC], u32, tag="dg")
                    nc.sync.dma_start(out=dwords[:], in_=words[0::16, :])
                    nc.gpsimd.collective_compute(
                        "AllGather", Alu.bypass,
                        replica_groups=[list(range(NC))],
                        ins=[dwords.opt()], outs=[dgath.opt()])
                    # T2N gather emitted after the collective trigger so it
                    # overlaps the AllGather on the TOPSP engines
                    nc.gpsimd.ap_gather(out_ap=qn[:], in_ap=T2N1[:],
                                        idxs_ap=idx16[:], channels=P,
                                        num_elems=T2SIZE, d=1,
                                        num_idxs=NODES_CORE)
                    nc.sync.dma_start(
                        out=W[:, 1:],
                        in_=dgath[0:1, :].broadcast_to([P, NC * WORDS_NC]))
            nc.sync.dma_start(out=t_qout[:], in_=q[:])
    nc.compile()
    return nc


def _device_inputs(lay, s0, T):
    ns_tab = np.argmax(np.asarray(T), axis=2).astype(np.uint32)  # [256, 8]
    flat = ns_tab.reshape(-1)  # idx-1 = mask*8 + state
    T2N1 = np.zeros(T2SIZE, dtype=np.uint32)
    T2N1[1:] = flat + 1
    T2L0 = np.zeros(T2SIZE, dtype=np.uint32)
    T2L0[1:] = (1 << flat).astype(np.uint32)

    st_node = np.argmax(np.asarray(s0), axis=1).astype(np.uint32)
    st_z = np.zeros(NTOT, dtype=np.uint32)
    st_z[:N_REAL] = st_node[lay.node_of_z[:N_REAL]]
    W0 = np.zeros(NWORDS, dtype=np.uint32)
    byte = (1 << st_z).astype(np.uint32) << (8 * lay.lane_of_z)
    np.bitwise_or.at(W0, lay.word_of_z, byte)

    stg = np.zeros((NC, P, JROWS), dtype=np.uint32)
    stg[lay.nc_of_z, lay.p_of_z, lay.j_of_z] = st_z
    lanecode = np.broadcast_to(((np.arange(RPC) % 4) * 8).astype(np.uint32), (P, RPC)).copy()
    m16 = np.zeros((P, RPC), dtype=np.uint32)
    m16[np.arange(P), np.arange(P) % RPC] = 0xFFFFFFFF
    W0b = np.broadcast_to(W0, (P, NWORDS)).copy()
    T2N1b = np.broadcast_to(T2N1, (P, T2SIZE)).copy()
    T2L0b = np.broadcast_to(T2L0, (P, T2SIZE)).copy()

    in_maps = []
    for nci in range(NC):
        q0 = np.zeros((P, NODES_CORE), dtype=np.uint32)
        shf = np.zeros((P, lay.slots_per_core), dtype=np.uint8)
        for c in range(CORES):
            sgrid = stg[nci, c * RPC:(c + 1) * RPC, :]
            q0[c * RPC:(c + 1) * RPC, :] = (sgrid.T.reshape(-1) + 1)[None, :]
            shf[c * RPC:(c + 1) * RPC, :] = lay.stream_sh[nci, c][None, :]
        in_maps.append({
            "t_idx": lay.idx_wrapped[nci],
            "t_shift": shf,
            "t_W0": W0b,
            "t_q0": q0,
            "t_T2N1": T2N1b,
            "t_T2L0": T2L0b,
            "t_lane": lanecode,
            "t_m16": m16,
        })
    return in_maps


def _decode(lay, results):
    stg = np.zeros((NC, P, JROWS), dtype=np.uint32)
    for nci in range(NC):
        qout = results[nci]["t_qout"]
        for c in range(CORES):
            stg[nci, c * RPC:(c + 1) * RPC, :] = \
                qout[c * RPC, :].reshape(JROWS, RPC).T
    st_z = stg[lay.nc_of_z[:N_REAL], lay.p_of_z[:N_REAL],
               lay.j_of_z[:N_REAL]].astype(np.int64) - 1
    st_node = np.zeros(N_REAL, dtype=np.int64)
    st_node[lay.node_of_z[:N_REAL]] = st_z
    out = np.zeros((N_REAL, S), dtype=np.float32)
    out[np.arange(N_REAL), st_node] = 1.0
    return out


def kernel(s0, edge_index, T):
    global LAST_EXEC_NS
    from concourse import bass_utils

    s0 = np.asarray(s0)
    edge_index = np.asarray(edge_index)
    Tn = np.asarray(T)
    lay = _Layout(edge_index)
    nc = _build_kernel(lay.chunks, lay.slots_per_core)
    in_maps = _device_inputs(lay, s0, Tn)
    trace = os.environ.get("BASS_FSM_TRACE", "0") == "1"
    res = bass_utils.run_bass_kernel_spmd(
        nc, in_maps, core_ids=list(range(NC)), trace=trace)
    LAST_EXEC_NS = res.exec_time_ns
    return _decode(lay, res.results).astype(s0.dtype)
